# revision 9
# baseline (speedup 1.0000x reference)
"""Trainium2 Bass kernel for: 3x3 conv (reflect pad) + BatchNorm + LeakyReLU + mask.

Input  x:    (1, 64, 512, 512) f32
       W:    (128, 64, 3, 3)   f32
       gamma/beta/mean/var: (128,) f32
       mask: (1, 128, 512, 512) int32 (0/1)
Output (1, 128, 512, 512) f32

Strategy (8 cores, SPMD):
  - Shard H spatially: core c computes output rows [64c, 64c+64).
  - Host reflect-pads x to (64, 514, 514), appends 2 duplicate rows, and ships
    each core TWO bf16 copies of its 67-row slab (second copy shifted down one
    row) stacked into a [128, 67*514] SBUF image. A K=128 matmul against
    stacked weights then computes two conv taps at once:
      partitions   0..63 : channel ci at row y+dy
      partitions 64..127 : channel ci at row y+dy+1
  - 9 taps -> 6 matmuls per output row: 3 "pair" matmuls (dy=0&1, dx=0..2)
    and 3 dy=2 matmuls whose lower 64 weight rows are zero.
  - PSUM accumulates fp32; epilogue = ACT Identity(psum*scale+shift),
    DVE max(z*0.01, z) for LeakyReLU, DVE multiply by uint8 mask.
"""

import numpy as np
import ml_dtypes

import concourse.bacc as bacc
import concourse.bass as bass
import concourse.mybir as mybir
import concourse.tile as tile
from concourse.bass_utils import run_bass_kernel_spmd

bf16 = ml_dtypes.bfloat16

N_CORES = 8
C_IN = 64
C_OUT = 128
H = 512
W_IMG = 512
HS = H // N_CORES            # 64 output rows per core
WP = W_IMG + 2               # 514 padded columns
NROW = HS + 3                # 67 rows per stacked copy
FREE = NROW * WP             # per-partition free elems of the x image
G = 8                        # output rows per pipeline group
LEAK = 0.01
EPS = 1e-5

_CACHE = {}
LAST_RESULTS = None          # BassKernelResults of the last run (for test.py)


def _build_program() -> bass.Bass:
    nc = bacc.Bacc("TRN2", target_bir_lowering=False, debug=False,
                   num_devices=N_CORES)
    f32 = mybir.dt.float32
    bf = mybir.dt.bfloat16
    u8 = mybir.dt.uint8

    xs_d = nc.dram_tensor("xs", [128, FREE], bf, kind="ExternalInput")
    wp_d = nc.dram_tensor("wp", [6, 128, C_OUT], bf, kind="ExternalInput")
    bn_d = nc.dram_tensor("bn", [C_OUT, 2], f32, kind="ExternalInput")
    mk_d = nc.dram_tensor("msk", [C_OUT, HS * W_IMG], u8, kind="ExternalInput")
    out_d = nc.dram_tensor("out", [C_OUT, HS * W_IMG], f32, kind="ExternalOutput")

    with tile.TileContext(nc) as tc:
        with tc.tile_pool(name="const", bufs=1) as cpool, \
             tc.tile_pool(name="xp", bufs=1) as xpool, \
             tc.tile_pool(name="mp", bufs=3) as mpool, \
             tc.tile_pool(name="zp", bufs=4) as zpool, \
             tc.tile_pool(name="op", bufs=3) as opool, \
             tc.tile_pool(name="ps", bufs=8, space="PSUM") as ppool:

            wts = []
            for j in range(6):
                wj = cpool.tile([128, C_OUT], bf, name=f"w{j}", tag=f"w{j}")
                nc.sync.dma_start(out=wj[:], in_=wp_d[j, :, :])
                wts.append(wj)
            bn = cpool.tile([C_OUT, 2], f32, name="bn_t", tag="bn_t")
            nc.sync.dma_start(out=bn[:], in_=bn_d[:])

            xs = xpool.tile([128, FREE], bf, name="xs_t", tag="xs_t")
            XCH = 8                      # padded rows per x DMA chunk (~1 MiB)
            for r0 in range(0, NROW, XCH):
                r1 = min(NROW, r0 + XCH)
                nc.sync.dma_start(out=xs[:, r0 * WP:r1 * WP],
                                  in_=xs_d[:, r0 * WP:r1 * WP])

            for g in range(HS // G):
                y0 = g * G
                mt = mpool.tile([C_OUT, G * W_IMG], u8, name="mt", tag="mt")
                nc.sync.dma_start(out=mt[:],
                                  in_=mk_d[:, y0 * W_IMG:(y0 + G) * W_IMG])
                ot = opool.tile([C_OUT, G * W_IMG], f32, name="ot", tag="ot")
                pst = [ppool.tile([C_OUT, W_IMG], f32, name=f"ps{yy}", tag="pst")
                       for yy in range(G)]
                for j in range(6):
                    dy = 0 if j < 3 else 2
                    dx = j % 3
                    for yy in range(G):
                        off = (y0 + yy + dy) * WP + dx
                        nc.tensor.matmul(pst[yy][:], wts[j][:],
                                         xs[:, off:off + W_IMG],
                                         start=(j == 0), stop=(j == 5))
                for yy in range(G):
                    seg = slice(yy * W_IMG, (yy + 1) * W_IMG)
                    zt = zpool.tile([C_OUT, W_IMG], f32, name="zt", tag="zt")
                    nc.scalar.activation(zt[:], pst[yy][:],
                                         mybir.ActivationFunctionType.Identity,
                                         bias=bn[:, 1:2], scale=bn[:, 0:1])
                    # LeakyReLU: max(z*0.01, z)
                    nc.vector.scalar_tensor_tensor(
                        ot[:, seg], zt[:], LEAK, zt[:],
                        op0=mybir.AluOpType.mult, op1=mybir.AluOpType.max)
                    # dropout-style 0/1 mask
                    nc.vector.tensor_tensor(ot[:, seg], ot[:, seg], mt[:, seg],
                                            op=mybir.AluOpType.mult)
                nc.sync.dma_start(out=out_d[:, y0 * W_IMG:(y0 + G) * W_IMG],
                                  in_=ot[:])
    nc.compile()
    return nc


def _get_program() -> bass.Bass:
    if "nc" not in _CACHE:
        _CACHE["nc"] = _build_program()
    return _CACHE["nc"]


def make_in_maps(x, W, gamma, beta, mean, var, mask):
    """Host-side shard/pack of full inputs into per-core in_maps."""
    x = np.asarray(x, np.float32)
    W = np.asarray(W, np.float32)
    gamma = np.asarray(gamma, np.float32)
    beta = np.asarray(beta, np.float32)
    mean = np.asarray(mean, np.float32)
    var = np.asarray(var, np.float32)
    mask = np.asarray(mask)

    xp = np.pad(x[0], ((0, 0), (1, 1), (1, 1)), mode="reflect")   # [64,514,514]
    xpe = np.concatenate([xp, np.repeat(xp[:, -1:, :], 2, axis=1)], axis=1)
    xpb = xpe.astype(bf16)                                        # [64,516,514]

    wp = np.zeros((6, 128, C_OUT), np.float32)
    for dx in range(3):
        wp[dx, 0:64] = W[:, :, 0, dx].reshape(C_OUT, C_IN).T
        wp[dx, 64:128] = W[:, :, 1, dx].reshape(C_OUT, C_IN).T
        wp[3 + dx, 0:64] = W[:, :, 2, dx].reshape(C_OUT, C_IN).T
    wp = wp.astype(bf16)

    inv = 1.0 / np.sqrt(var + EPS)
    bn = np.stack([gamma * inv, beta - mean * gamma * inv],
                  axis=1).astype(np.float32)                      # [128,2]

    m8 = mask[0].astype(np.uint8)                                 # [128,512,512]

    in_maps = []
    for c in range(N_CORES):
        S = xpb[:, HS * c:HS * c + HS + 4, :]
        copy0 = np.ascontiguousarray(S[:, 0:NROW, :]).reshape(C_IN, FREE)
        copy1 = np.ascontiguousarray(S[:, 1:NROW + 1, :]).reshape(C_IN, FREE)
        xs_c = np.concatenate([copy0, copy1], axis=0)             # [128, FREE]
        mk_c = np.ascontiguousarray(
            m8[:, HS * c:HS * c + HS, :]).reshape(C_OUT, HS * W_IMG)
        in_maps.append(dict(xs=xs_c, wp=wp, bn=bn, msk=mk_c))
    return in_maps


def kernel(x, W, gamma, beta, mean, var, mask, _trace=False):
    global LAST_RESULTS
    nc = _get_program()
    in_maps = make_in_maps(x, W, gamma, beta, mean, var, mask)
    res = run_bass_kernel_spmd(nc, in_maps, list(range(N_CORES)), trace=_trace)
    LAST_RESULTS = res
    out = np.empty((1, C_OUT, H, W_IMG), np.float32)
    for c in range(N_CORES):
        out[0, :, HS * c:HS * c + HS, :] = \
            np.asarray(res.results[c]["out"]).reshape(C_OUT, HS, W_IMG)
    return out


# revision 12
# speedup vs baseline: 1.0154x; 1.0154x over previous
"""Trainium2 Bass kernel for: 3x3 conv (reflect pad) + BatchNorm + LeakyReLU + mask.

Input  x:    (1, 64, 512, 512) f32
       W:    (128, 64, 3, 3)   f32
       gamma/beta/mean/var: (128,) f32
       mask: (1, 128, 512, 512) int32 (0/1)
Output (1, 128, 512, 512) f32

Strategy (8 cores, SPMD):
  - Shard H spatially: core c computes output rows [64c, 64c+64).
  - Host reflect-pads x to (64, 514, 514), appends 2 duplicate rows, and ships
    each core TWO bf16 copies of its 67-row slab (second copy shifted down one
    row) stacked into a [128, 67*514] SBUF image. A K=128 matmul against
    stacked weights then computes two conv taps at once:
      partitions   0..63 : channel ci at row y+dy
      partitions 64..127 : channel ci at row y+dy+1
  - 9 taps -> 6 matmuls per output row: 3 "pair" matmuls (dy=0&1, dx=0..2)
    and 3 dy=2 matmuls whose lower 64 weight rows are zero.
  - PSUM accumulates fp32; epilogue = ACT Identity(psum*scale+shift),
    DVE max(z*0.01, z) for LeakyReLU, DVE multiply by uint8 mask.
"""

import numpy as np
import ml_dtypes

import concourse.bacc as bacc
import concourse.bass as bass
import concourse.mybir as mybir
import concourse.tile as tile
from concourse.bass_utils import run_bass_kernel_spmd

bf16 = ml_dtypes.bfloat16

N_CORES = 8
C_IN = 64
C_OUT = 128
H = 512
W_IMG = 512
HS = H // N_CORES            # 64 output rows per core
WP = W_IMG + 2               # 514 padded columns
NROW = HS + 3                # 67 rows per stacked copy
FREE = NROW * WP             # per-partition free elems of the x image
G = 8                        # output rows per pipeline group
LEAK = 0.01
EPS = 1e-5

_CACHE = {}
LAST_RESULTS = None          # BassKernelResults of the last run (for test.py)


def _build_program(hw_lrelu: bool = True) -> bass.Bass:
    """hw_lrelu=True uses the ACT engine's native Lrelu (not implemented in
    CoreSim); False uses an Identity + DVE max(z*a, z) fallback."""
    nc = bacc.Bacc("TRN2", target_bir_lowering=False, debug=False,
                   num_devices=N_CORES)
    f32 = mybir.dt.float32
    bf = mybir.dt.bfloat16
    u8 = mybir.dt.uint8

    xs_d = nc.dram_tensor("xs", [128, FREE], bf, kind="ExternalInput")
    wp_d = nc.dram_tensor("wp", [6, 128, C_OUT], bf, kind="ExternalInput")
    bn_d = nc.dram_tensor("bn", [C_OUT, 2], f32, kind="ExternalInput")
    mk_d = nc.dram_tensor("msk", [C_OUT, HS * W_IMG], u8, kind="ExternalInput")
    out_d = nc.dram_tensor("out", [C_OUT, HS * W_IMG], f32, kind="ExternalOutput")

    with tile.TileContext(nc) as tc:
        with tc.tile_pool(name="const", bufs=1) as cpool, \
             tc.tile_pool(name="xp", bufs=1) as xpool, \
             tc.tile_pool(name="mp", bufs=3) as mpool, \
             tc.tile_pool(name="zp", bufs=4) as zpool, \
             tc.tile_pool(name="op", bufs=3) as opool, \
             tc.tile_pool(name="ps", bufs=8, space="PSUM") as ppool:

            wts = []
            for j in range(6):
                wj = cpool.tile([128, C_OUT], bf, name=f"w{j}", tag=f"w{j}")
                nc.sync.dma_start(out=wj[:], in_=wp_d[j, :, :])
                wts.append(wj)
            bn = cpool.tile([C_OUT, 2], f32, name="bn_t", tag="bn_t")
            nc.sync.dma_start(out=bn[:], in_=bn_d[:])

            xs = xpool.tile([128, FREE], bf, name="xs_t", tag="xs_t")
            # fine chunks up front so PE can start early, 8-row after
            xrow = [0, 3, 7, 11, 19, 27, 35, 43, 51, 59, NROW]
            for r0, r1 in zip(xrow[:-1], xrow[1:]):
                nc.sync.dma_start(out=xs[:, r0 * WP:r1 * WP],
                                  in_=xs_d[:, r0 * WP:r1 * WP])

            SG = 4                       # output rows per store
            for g in range(HS // G):
                y0 = g * G
                mt = mpool.tile([C_OUT, G * W_IMG], u8, name="mt", tag="mt")
                nc.sync.dma_start(out=mt[:],
                                  in_=mk_d[:, y0 * W_IMG:(y0 + G) * W_IMG])
                pst = [ppool.tile([C_OUT, W_IMG], f32, name=f"ps{yy}", tag="pst")
                       for yy in range(G)]
                for j in range(6):
                    dy = 0 if j < 3 else 2
                    dx = j % 3
                    for yy in range(G):
                        off = (y0 + yy + dy) * WP + dx
                        nc.tensor.matmul(pst[yy][:], wts[j][:],
                                         xs[:, off:off + W_IMG],
                                         start=(j == 0), stop=(j == 5))
                for s0 in range(0, G, SG):
                    ot = opool.tile([C_OUT, SG * W_IMG], f32, name="ot", tag="ot")
                    for k in range(SG):
                        yy = s0 + k
                        seg = slice(k * W_IMG, (k + 1) * W_IMG)
                        mseg = slice(yy * W_IMG, (yy + 1) * W_IMG)
                        if hw_lrelu:
                            nc.scalar.activation(
                                ot[:, seg], pst[yy][:],
                                mybir.ActivationFunctionType.Lrelu,
                                bias=bn[:, 1:2], scale=bn[:, 0:1], alpha=LEAK)
                        else:
                            zt = zpool.tile([C_OUT, W_IMG], f32,
                                            name="zt", tag="zt")
                            nc.scalar.activation(
                                zt[:], pst[yy][:],
                                mybir.ActivationFunctionType.Identity,
                                bias=bn[:, 1:2], scale=bn[:, 0:1])
                            nc.vector.scalar_tensor_tensor(
                                ot[:, seg], zt[:], LEAK, zt[:],
                                op0=mybir.AluOpType.mult,
                                op1=mybir.AluOpType.max)
                        nc.vector.tensor_tensor(ot[:, seg], ot[:, seg],
                                                mt[:, mseg],
                                                op=mybir.AluOpType.mult)
                    d0 = (y0 + s0) * W_IMG
                    nc.sync.dma_start(out=out_d[:, d0:d0 + SG * W_IMG],
                                      in_=ot[:])
    nc.compile()
    return nc


def _get_program(hw_lrelu: bool = True) -> bass.Bass:
    key = ("nc", hw_lrelu)
    if key not in _CACHE:
        _CACHE[key] = _build_program(hw_lrelu)
    return _CACHE[key]


def make_in_maps(x, W, gamma, beta, mean, var, mask):
    """Host-side shard/pack of full inputs into per-core in_maps."""
    x = np.asarray(x, np.float32)
    W = np.asarray(W, np.float32)
    gamma = np.asarray(gamma, np.float32)
    beta = np.asarray(beta, np.float32)
    mean = np.asarray(mean, np.float32)
    var = np.asarray(var, np.float32)
    mask = np.asarray(mask)

    xp = np.pad(x[0], ((0, 0), (1, 1), (1, 1)), mode="reflect")   # [64,514,514]
    xpe = np.concatenate([xp, np.repeat(xp[:, -1:, :], 2, axis=1)], axis=1)
    xpb = xpe.astype(bf16)                                        # [64,516,514]

    wp = np.zeros((6, 128, C_OUT), np.float32)
    for dx in range(3):
        wp[dx, 0:64] = W[:, :, 0, dx].reshape(C_OUT, C_IN).T
        wp[dx, 64:128] = W[:, :, 1, dx].reshape(C_OUT, C_IN).T
        wp[3 + dx, 0:64] = W[:, :, 2, dx].reshape(C_OUT, C_IN).T
    wp = wp.astype(bf16)

    inv = 1.0 / np.sqrt(var + EPS)
    bn = np.stack([gamma * inv, beta - mean * gamma * inv],
                  axis=1).astype(np.float32)                      # [128,2]

    m8 = mask[0].astype(np.uint8)                                 # [128,512,512]

    in_maps = []
    for c in range(N_CORES):
        S = xpb[:, HS * c:HS * c + HS + 4, :]
        copy0 = np.ascontiguousarray(S[:, 0:NROW, :]).reshape(C_IN, FREE)
        copy1 = np.ascontiguousarray(S[:, 1:NROW + 1, :]).reshape(C_IN, FREE)
        xs_c = np.concatenate([copy0, copy1], axis=0)             # [128, FREE]
        mk_c = np.ascontiguousarray(
            m8[:, HS * c:HS * c + HS, :]).reshape(C_OUT, HS * W_IMG)
        in_maps.append(dict(xs=xs_c, wp=wp, bn=bn, msk=mk_c))
    return in_maps


def kernel(x, W, gamma, beta, mean, var, mask, _trace=False):
    global LAST_RESULTS
    nc = _get_program()
    in_maps = make_in_maps(x, W, gamma, beta, mean, var, mask)
    res = run_bass_kernel_spmd(nc, in_maps, list(range(N_CORES)), trace=_trace)
    LAST_RESULTS = res
    out = np.empty((1, C_OUT, H, W_IMG), np.float32)
    for c in range(N_CORES):
        out[0, :, HS * c:HS * c + HS, :] = \
            np.asarray(res.results[c]["out"]).reshape(C_OUT, HS, W_IMG)
    return out
